# revision 39
# baseline (speedup 1.0000x reference)
"""Trainium2 Bass kernel for nn_Attention (B=4,T=2048,C=512,H=8 causal RoPE attention).

Sharding: 8 cores = 4 batches x 2 head-groups. Core c handles batch c//2 and
heads [4*(c%2), 4*(c%2)+4). Each core computes its proj partial y_part[T, C];
the host sums the two partials per batch and adds bp.

Key HW facts this kernel is shaped around (microbenched):
  - a matmul with 128 contraction partitions costs ~263ns at N=512; one with
    64 or 32 partitions costs ~471ns regardless of dtype. So every matmul
    here uses K=128 partitions.
  - fp8 DoubleRow with [128,2,*] operands contracts 256 channels at the same
    ~270ns -> Q/K projections (fp8 is accuracy-safe there) halve their count.
  - matmul N caps at 512 (one PSUM bank).

Per-core dataflow:
  Q/K projections: fp8(e4m3) DoubleRow, host-reordered channels (m-tile 'e' =
  even RoPE-pair members at row 32h+i, 'o' = odd). RoPE runs on DVE straight
  from PSUM: the pair partner sits at the same partition of the other m-tile,
  so no partition swap is needed. The rotated [128,2,512] result is then
  DMA-scattered (DMA engines are idle) into:
    qTd[h] [128, T]: head h dims 0..63 on rows 0-63 AND duplicated on 64-127.
    kbd[h] [128, T]: block-diagonal K: col c (= kt position), rows 0-63 hold
      k dims when (c%128)<64, rows 64-127 when >=64, zeros elsewhere.
  Scores for one [kt=128 x q=512] tile are then ONE K=128 bf16 matmul
  (kbd[h][:,128it:+128].T @ qTd[h][:,qsl]) - two kt-64 blocks per matmul.
  exp on ACT (scale=1/8 folded, no max subtraction - scores are O(1)), causal
  handled by trimming fully-masked leading columns of diagonal tiles in
  scores/exp/PV plus one [128,128] triangular mask multiply per diagonal
  tile. PV accumulates bf16 with a ones-column row giving the softmax
  denominator; normalization via ones-broadcast matmul + reciprocal +
  multiply. Output projection bf16, PSUM->SBUF copy, DMA out.
"""

import sys

for _p in ("/opt/trn_rl_repo",):
    if _p not in sys.path:
        sys.path.insert(0, _p)

from contextlib import ExitStack

import ml_dtypes
import numpy as np

import concourse.bass as bass
import concourse.tile as tile
from concourse import bacc
from concourse import mybir
from concourse.bass_utils import run_bass_kernel_spmd


def _ensure_ntff_hook():
    """Provide antenv.axon_hooks (missing in this image) so trace=True works."""
    try:
        import antenv.axon_hooks  # noqa: F401

        return
    except ImportError:
        pass
    import contextlib
    import ctypes
    import types

    import antenv

    mod = types.ModuleType("antenv.axon_hooks")
    holder = {}
    mod.set_axon_ntff_profile_hook = lambda h: holder.__setitem__("h", h)
    mod.get_axon_ntff_profile_hook = lambda: holder.get("h")
    antenv.axon_hooks = mod
    sys.modules["antenv.axon_hooks"] = mod

    so_path = "/opt/axon/libaxon_pjrt.so"
    try:
        lib = ctypes.CDLL(so_path)
    except OSError:
        return
    if not hasattr(lib, "axon_start_nrt_profile"):
        return
    lib.axon_start_nrt_profile.argtypes = [
        ctypes.POINTER(ctypes.c_int64),
        ctypes.c_size_t,
    ]
    lib.axon_start_nrt_profile.restype = ctypes.c_int64
    lib.axon_stop_nrt_profile.argtypes = [ctypes.c_char_p]
    lib.axon_stop_nrt_profile.restype = ctypes.c_int64

    @contextlib.contextmanager
    def _hook(output_dir, device_ids):
        import jax

        jax.devices()
        if device_ids:
            ids = (ctypes.c_int64 * len(device_ids))(*device_ids)
            rc = lib.axon_start_nrt_profile(ids, len(device_ids))
        else:
            rc = lib.axon_start_nrt_profile(None, 0)
        if rc != 0:
            raise RuntimeError(f"axon_start_nrt_profile rc={rc}")
        try:
            yield
        finally:
            n = lib.axon_stop_nrt_profile(str(output_dir).encode())
            print(f"profile: {n} file(s) written to {output_dir}", file=sys.stderr)

    mod.set_axon_ntff_profile_hook(_hook)

BF16 = mybir.dt.bfloat16
F32 = mybir.dt.float32
FP8 = mybir.dt.float8e4
NPBF = ml_dtypes.bfloat16
NPF8 = ml_dtypes.float8_e4m3fn

B, C, H, D = 4, 512, 8, 64
HPC = 4              # heads per core
CL = HPC * D         # 256 local channels
NCORES = 8
THETA = 10000.0
QC = 512             # q-chunk width (free dim per matmul)
GROUP = 2            # kt-tiles per exp group (PSUM banks per score tile)
ACT_EXP = mybir.ActivationFunctionType.Exp
DR = mybir.MatmulPerfMode.DoubleRow


def build_nc(T: int) -> bass.Bass:
    PT = T // 128
    NJ = T // QC
    nc = bacc.Bacc()

    xT = nc.declare_dram_parameter("xT", [C, T], BF16, isOutput=False)
    xT8 = nc.declare_dram_parameter("xT8", [128, 4 * T], FP8, isOutput=False)
    wq8 = nc.declare_dram_parameter("wq8", [128, 1024], FP8, isOutput=False)
    wk8 = nc.declare_dram_parameter("wk8", [128, 1024], FP8, isOutput=False)
    wv = nc.declare_dram_parameter("wv", [C, CL], BF16, isOutput=False)
    wp = nc.declare_dram_parameter("wp", [CL, C], BF16, isOutput=False)
    csb = nc.declare_dram_parameter("csb", [128, T], BF16, isOutput=False)
    snb = nc.declare_dram_parameter("snb", [128, T], BF16, isOutput=False)
    msk = nc.declare_dram_parameter("msk", [128, 128], BF16, isOutput=False)
    y = nc.declare_dram_parameter("y", [T, C], BF16, isOutput=True)

    with nc.allow_low_precision(
        reason="bf16/fp8 compute by design; f32 PSUM accumulation everywhere"
    ), tile.TileContext(nc) as tc, ExitStack() as ctx:
        pers = ctx.enter_context(tc.tile_pool(name="pers", bufs=1))
        work = ctx.enter_context(tc.tile_pool(name="work", bufs=2))
        pexp = ctx.enter_context(tc.tile_pool(name="pexp", bufs=5))
        big = ctx.enter_context(tc.tile_pool(name="big", bufs=2, space="PSUM"))
        b1 = ctx.enter_context(tc.tile_pool(name="b1", bufs=4, space="PSUM"))

        # ---------------- persistent SBUF: inputs ----------------
        xT_sb = [pers.tile([128, T], BF16, name=f"xT{i}", tag=f"xT{i}") for i in range(4)]
        # [p, a(kc-pair), u(half), t]: in-chan = 256a + 128u + p
        xT8_sb = pers.tile([128, 2, 2, T], FP8, name="xT8", tag="xT8")
        # [p, mt(e/o), a, u, mout]
        wq8_sb = pers.tile([128, 2, 2, 2, 128], FP8, name="wq8", tag="wq8")
        wk8_sb = pers.tile([128, 2, 2, 2, 128], FP8, name="wk8", tag="wk8")
        wv_sb = [pers.tile([128, CL], BF16, name=f"wv{i}", tag=f"wv{i}") for i in range(4)]
        wp_sb = [pers.tile([128, C], BF16, name=f"wp{i}", tag=f"wp{i}") for i in range(2)]
        cs_sb = pers.tile([128, T], BF16, name="cs", tag="cs")
        sn_sb = pers.tile([128, T], BF16, name="sn", tag="sn")
        msk_sb = pers.tile([128, 128], BF16, name="msk", tag="msk")

        # qk-proj inputs first (they gate the whole pipeline), split so the
        # transfers parallelize across DMA queues
        nc.sync.dma_start(out=wq8_sb[:], in_=wq8[:, :])
        nc.sync.dma_start(out=wk8_sb[:], in_=wk8[:, :])
        for a in range(2):
            for u in range(2):
                nc.sync.dma_start(
                    out=xT8_sb[:, a, u, :],
                    in_=xT8[:, (2 * a + u) * T:(2 * a + u + 1) * T],
                )
        nc.sync.dma_start(out=cs_sb[:], in_=csb[:, :])
        nc.sync.dma_start(out=sn_sb[:], in_=snb[:, :])
        nc.sync.dma_start(out=msk_sb[:], in_=msk[:, :])
        for i in range(4):
            nc.sync.dma_start(out=xT_sb[i][:], in_=xT[128 * i:128 * i + 128, :])
            nc.sync.dma_start(out=wv_sb[i][:], in_=wv[128 * i:128 * i + 128, :])
        for i in range(2):
            nc.sync.dma_start(out=wp_sb[i][:], in_=wp[128 * i:128 * i + 128, :])

        # ---------------- persistent SBUF: intermediates ----------------
        # qTd[h]: head h dims (natural order) on rows 0-63, duplicated 64-127
        qTd = [pers.tile([128, T], BF16, name=f"qTd{h}", tag=f"qTd{h}") for h in range(HPC)]
        # kbd[h]: block-diagonal k (see module docstring); zeros elsewhere
        kbd = [pers.tile([128, T], BF16, name=f"kbd{h}", tag=f"kbd{h}") for h in range(HPC)]
        for h in range(HPC):
            nc.vector.memset(kbd[h][:], 0.0)
        # [kt-part, head, d] with col 64 = ones (denominator row)
        vx_sb = [pers.tile([128, HPC, D + 1], BF16, name=f"vx{i}", tag=f"vx{i}") for i in range(PT)]
        rnT_sb = [pers.tile([128, T], BF16, name=f"rn{i}", tag=f"rn{i}") for i in range(2)]
        ones_sb = pers.tile([1, 64], BF16, name="ones", tag="ones")
        nc.vector.memset(ones_sb[:], 1.0)

        def vproj(tt):
            pv = b1.tile([128, 512], F32, name="b1", tag="b1")
            for kc in range(4):
                nc.tensor.matmul(
                    pv[:, 0:CL],
                    lhsT=xT_sb[kc][:, 128 * tt:128 * tt + 128],
                    rhs=wv_sb[kc][:],
                    start=(kc == 0),
                    stop=(kc == 3),
                )
            # middle tiles ride the still-idle ACT; others go to DVE
            if 4 <= tt < 12:
                nc.scalar.copy(
                    vx_sb[tt][:, :, 0:64],
                    pv[:, 0:CL].rearrange("p (h d) -> p h d", h=HPC),
                )
            else:
                nc.vector.tensor_copy(
                    vx_sb[tt][:, :, 0:64],
                    pv[:, 0:CL].rearrange("p (h d) -> p h d", h=HPC),
                )
            nc.gpsimd.memset(vx_sb[tt][:, :, 64:65], 1.0)

        def qkproj(t4):
            tsl = slice(QC * t4, QC * t4 + QC)
            for w8, dstq, dstk in ((wk8_sb, None, kbd), (wq8_sb, qTd, None)):
                pq = big.tile([128, GROUP * 512], F32, name="big", tag="big")
                for mt in range(2):
                    for a in range(2):
                        nc.tensor.matmul(
                            pq[:, 512 * mt:512 * mt + 512],
                            lhsT=w8[:, mt, a],
                            rhs=xT8_sb[:, a, :, tsl],
                            start=(a == 0),
                            stop=(a == 1),
                            perf_mode=DR,
                        )
                # PSUM->SBUF bf16 copy on ACT (idle early); the DVE rope ops
                # then run all-SBUF/2-byte, hitting the 2x DVE mode
                pqc = work.tile([128, 2, 512], BF16, name="pqc", tag="pqc", bufs=2)
                nc.scalar.copy(pqc[:], pq[:, 0:1024].rearrange("p (u t) -> p u t", u=2))
                ta = work.tile([128, 512], BF16, name="ta", tag="ta", bufs=2)
                tb = work.tile([128, 512], BF16, name="tb", tag="tb", bufs=2)
                tc_ = work.tile([128, 512], BF16, name="tc", tag="tc", bufs=2)
                td = work.tile([128, 512], BF16, name="td", tag="td", bufs=2)
                rot = work.tile([128, 2, 512], BF16, name="rot", tag="rot", bufs=4)
                nc.vector.tensor_mul(ta[:], pqc[:, 0, :], cs_sb[:, tsl])
                nc.vector.tensor_mul(tb[:], pqc[:, 1, :], sn_sb[:, tsl])
                nc.vector.tensor_mul(tc_[:], pqc[:, 0, :], sn_sb[:, tsl])
                nc.vector.tensor_mul(td[:], pqc[:, 1, :], cs_sb[:, tsl])
                nc.vector.tensor_sub(rot[:, 0, :], ta[:], tb[:])
                nc.vector.tensor_add(rot[:, 1, :], tc_[:], td[:])
                for h in range(HPC):
                    rsl = rot[32 * h:32 * h + 32, :, :]
                    if dstq is not None:
                        nc.sync.dma_start(out=dstq[h][0:64, tsl], in_=rsl)
                        nc.sync.dma_start(out=dstq[h][64:128, tsl], in_=rsl)
                    else:
                        rblk = rsl.rearrange("p u (b c) -> p u b c", b=4)
                        ksl = dstk[h][:, tsl].rearrange("p (b c) -> p b c", b=4)
                        nc.gpsimd.dma_start(
                            out=ksl[0:64, :, 0:64], in_=rblk[:, :, :, 0:64]
                        )
                        nc.gpsimd.dma_start(
                            out=ksl[64:128, :, 64:128], in_=rblk[:, :, :, 64:128]
                        )

        # ---------------- stage C helpers ----------------
        # Software pipelining across ladders: each ladder's LAST PV group and
        # its normalization are deferred into the next ladder, emitted right
        # after that ladder's first score group. The PE then has real work
        # (prev PV + norm bcast) to chew on while the new ladder's first exp
        # group is still in flight, killing the ladder-boundary bubble.
        pending_tail = []
        pending_norm = []

        def flush_tail():
            while pending_tail:
                pending_tail.pop(0)()

        def flush_norm():
            while pending_norm:
                pending_norm.pop(0)()

        def norm_emit(ph, sub, j, pvp):
            qsl = slice(QC * j, QC * j + QC)
            den = work.tile([1, 512], BF16, name="den", tag="den", bufs=3)
            nc.vector.tensor_copy(den[:], pvp[64:65, :])

            def emit():
                bc = b1.tile([128, 512], F32, name="b1", tag="b1")
                nc.tensor.matmul(
                    bc[0:64, :],
                    lhsT=ones_sb[0:1, :],
                    rhs=den[0:1, :],
                    start=True,
                    stop=True,
                )
                rbc = work.tile([64, 512], F32, name="rbc", tag="rbc", bufs=2)
                nc.vector.reciprocal_approx_fast(rbc[:], bc[0:64, :])
                nc.vector.tensor_mul(
                    rnT_sb[ph][64 * sub:64 * sub + 64, qsl],
                    pvp[0:64, :],
                    rbc[:],
                )

            pending_norm.append(emit)

        def attn(ph, sub, j):
            h = 2 * ph + sub
            nkt = 4 * (j + 1)
            pvp = b1.tile([128, 512], F32, name="b1", tag="b1")
            groups = [
                list(range(u0, min(u0 + GROUP, nkt)))
                for u0 in range(0, nkt, GROUP)
            ]
            pv_queue = []
            for gi, grp in enumerate(groups):
                sg = big.tile([128, GROUP * 512], F32, name="big", tag="big")
                offs = []
                for ui, it in enumerate(grp):
                    r = it - 4 * j
                    qoff = 128 * r if r > 0 else 0
                    offs.append(qoff)
                    nc.tensor.matmul(
                        sg[:, 512 * ui + qoff:512 * ui + 512],
                        lhsT=kbd[h][:, 128 * it:128 * it + 128],
                        rhs=qTd[h][:, QC * j + qoff:QC * j + QC],
                        start=True,
                        stop=True,
                    )
                if gi == 0:
                    flush_tail()
                elif gi == min(2, len(groups) - 1):
                    flush_norm()
                # emit PV two score groups behind, so each exp has two
                # groups of PE work to hide under
                while len(pv_queue) >= 2:
                    pv_queue.pop(0)()
                pg = pexp.tile([128, GROUP * 512], BF16, name="pg", tag="pg")
                diag = grp[0] >= 4 * j
                if diag:
                    for ui, it in enumerate(grp):
                        qoff = offs[ui]
                        nc.scalar.activation(
                            pg[:, 512 * ui + qoff:512 * ui + 512],
                            sg[:, 512 * ui + qoff:512 * ui + 512],
                            ACT_EXP,
                            scale=0.125,
                        )
                        nc.vector.tensor_mul(
                            pg[:, 512 * ui + qoff:512 * ui + qoff + 128],
                            pg[:, 512 * ui + qoff:512 * ui + qoff + 128],
                            msk_sb[:],
                        )
                else:
                    w = 512 * len(grp)
                    nc.scalar.activation(
                        pg[:, 0:w], sg[:, 0:w], ACT_EXP, scale=0.125
                    )

                def pv_emit(grp=grp, offs=offs, pg=pg):
                    for ui, it in enumerate(grp):
                        qoff = offs[ui]
                        nc.tensor.matmul(
                            pvp[0:65, qoff:512],
                            lhsT=vx_sb[it][:, h, :],
                            rhs=pg[:, 512 * ui + qoff:512 * ui + 512],
                            start=(it == 0),
                            stop=(it == nkt - 1),
                            skip_group_check=True,
                        )

                pv_queue.append(pv_emit)
            while len(pv_queue) > 1:
                pv_queue.pop(0)()
            final_pv = pv_queue.pop(0)
            pending_tail.append(
                lambda: (final_pv(), norm_emit(ph, sub, j, pvp))
            )

        def outproj(tt, tail=False):
            ppt = big.tile([128, GROUP * 512], F32, name="big", tag="big")
            pp = ppt[:, 0:512]
            for kc in range(2):
                nc.tensor.matmul(
                    pp[:],
                    lhsT=rnT_sb[kc][:, 128 * tt:128 * tt + 128],
                    rhs=wp_sb[kc][:],
                    start=(kc == 0),
                    stop=(kc == 1),
                )
            ys = work.tile([128, 512], BF16, name="ys", tag="ys", bufs=3)
            nc.vector.tensor_copy(ys[:], pp[:])
            nc.sync.dma_start(out=y[128 * tt:128 * tt + 128, :], in_=ys[:])

        # ---------------- schedule ----------------
        # qk-proj chunks lead (their DVE-rope + DMA-scatter chains are the
        # long pole early on); v-proj fills the PE while chunk-0's chain
        # drains. Attention ladders then chase the per-chunk chains.
        qkproj(0)
        for tt in range(min(4, PT)):
            vproj(tt)
        if NJ > 1:
            qkproj(1)
        for tt in range(min(4, PT), PT):
            vproj(tt)
        attn(0, 0, 0)
        for j in range(1, NJ):
            if j + 1 < NJ:
                qkproj(j + 1)
            attn(0, 0, j)
        for j in range(NJ):
            attn(0, 1, j)
        for j in range(NJ):
            attn(1, 0, j)
        for j in range(NJ):
            attn(1, 1, j)
            # stage D for chunk j-1 becomes ready once norm(1,1,j-1) has
            # been flushed (inside this attn's first group); emit it here so
            # the tail after the last ladder stays short
            if j > 0:
                for tt in range(4 * (j - 1), 4 * j):
                    outproj(tt)
        flush_tail()
        flush_norm()
        for tt in range(4 * (NJ - 1), 4 * NJ):
            outproj(tt, tail=True)

    nc.finalize()
    return nc


def prep_core_inputs(x, Wq, Wk, Wv, Wp, core, T):
    b, g = core // 2, core % 2
    sl = slice(CL * g, CL * g + CL)
    Wq_loc = Wq[sl, :]  # [256 out, 512 in]
    Wk_loc = Wk[sl, :]
    # channel reorder: m-tile 'e' = even dims (2i) of head h at row 32h+i,
    # m-tile 'o' = odd dims (2i+1)
    hh = np.repeat(np.arange(HPC), 32)
    ii = np.tile(np.arange(32), HPC)
    perm_e = 64 * hh + 2 * ii
    perm_o = perm_e + 1

    def w8pack(W_loc):
        # [p, mt, a, u, mout] with in-chan = 256a + 128u + p
        out = np.empty((128, 2, 2, 2, 128), np.float32)
        for mt, perm in enumerate((perm_e, perm_o)):
            Wl = W_loc[perm, :].T  # [512 in, 128 out]
            out[:, mt] = Wl.reshape(2, 2, 128, 128).transpose(2, 0, 1, 3)
        return out.reshape(128, 1024)

    xb = np.ascontiguousarray(x[b].T).astype(np.float32)  # [C, T]
    xT8 = xb.reshape(2, 2, 128, T).transpose(2, 0, 1, 3).reshape(128, 4 * T)

    # RoPE tables: row 32h+i <-> global pair index 128g + 32h + i
    pg = 128 * g + np.arange(128)
    invf = THETA ** (-(2.0 * pg) / C)
    ang = np.arange(T)[None, :] * invf[:, None]
    cos = np.cos(ang).astype(np.float32)
    sin = np.sin(ang).astype(np.float32)
    

    # triangular mask for the partial diagonal block: valid iff qq >= p
    p = np.arange(128)[:, None]
    qq = np.arange(128)[None, :]
    m = (qq >= p).astype(np.float32)

    return {
        "xT": xb.astype(NPBF),
        "xT8": xT8.astype(NPF8),
        "wq8": w8pack(Wq_loc).astype(NPF8),
        "wk8": w8pack(Wk_loc).astype(NPF8),
        "wv": np.ascontiguousarray(Wv[sl, :].T).astype(NPBF),
        "wp": np.ascontiguousarray(Wp[:, sl].T).astype(NPBF),
        "csb": cos.astype(NPBF),
        "snb": sin.astype(NPBF),
        "msk": m.astype(NPBF),
    }


_NC_CACHE = {}


def _get_nc(T):
    if T not in _NC_CACHE:
        _NC_CACHE[T] = build_nc(T)
    return _NC_CACHE[T]


def kernel(x, Wq, Wk, Wv, Wp, bp, _trace=False):
    x = np.asarray(x, dtype=np.float32)
    Wq = np.asarray(Wq, dtype=np.float32)
    Wk = np.asarray(Wk, dtype=np.float32)
    Wv = np.asarray(Wv, dtype=np.float32)
    Wp = np.asarray(Wp, dtype=np.float32)
    bp = np.asarray(bp, dtype=np.float32)
    T = x.shape[1]
    nc = _get_nc(T)
    in_maps = [prep_core_inputs(x, Wq, Wk, Wv, Wp, c, T) for c in range(NCORES)]
    if _trace:
        _ensure_ntff_hook()
    res = run_bass_kernel_spmd(nc, in_maps, list(range(NCORES)), trace=_trace)
    out = np.zeros((B, T, C), np.float32)
    for b in range(B):
        out[b] = res.results[2 * b]["y"].astype(np.float32) + res.results[
            2 * b + 1
        ]["y"].astype(np.float32)
    out += bp[None, None, :]
    if _trace:
        return out, res
    return out
